# revision 59
# baseline (speedup 1.0000x reference)
"""DGCNN (nn_DGCNN_param_57904749085240) Trainium2 Bass kernel, v3.

Data-parallel over batch: 8 cores x 2 point clouds each, no collectives.

Per EdgeConv layer: W @ [x_j - x_i; x_i] = A x_j + Cc x_i (+t), folded with
eval-BN scale; max over knn commutes with the positive per-channel scale:
    y = leaky( max_{j in knn(i)} (A x_j) + Cc x_i + t )

knn via packed-integer top-k: pd_ps = G - xx_j/2 accumulated on PE (fp32 for
layer0, fp16 G for layers 1-3 with an fp32 aug row), then on ACT
q = int32(round(pd_ps * 2S)), one DVE STT packs ((q << 10) | col_index), and
per-128-column octant Max8 + a 64-wide merge (Max8/MatchReplace rounds) gives
the top-20 with indices carried in the low 10 bits.

v3 structural changes vs v2:
- conv epilogue: neighbor-max m (fp16) is accumulated into the Cc-psum via
  an identity matmul on PE and ACT adds the bias during the psum->SBUF fp16
  copy, so DVE only runs the leaky STT (removes the DVE tensor-tensor add).
- lc conv: ACT Identity folds the bias into the psum->SBUF fp16 copy; the
  DVE leaky STT carries accum_out for the mean pool.
(Notes for future sessions: AF.Lrelu ignores alpha on HW (acts as Relu) and
AF.Prelu wedges the device; gpsimd cannot run TensorScalarPtr/STT ops (the
neuronxcc engine check rejects them, though the cost model accepts); fp32r
matmul inputs must be produced by fp32r-rounding producers; Max8 on int32
goes through fp32 so packed keys must stay under 2^24; gpsimd
indirect_dma_start supports only ONE offset per partition on HW (k=1 exact,
k=20 returns garbage, [1,n] flat offset lists wedge the device) so a
20-neighbor row-gather-into-partitions scheme (556us in sim) is not
realizable; emitting l2/l3 knn+convs el-major instead of t-major interleaved
regresses (596us) - the scheduler's own interleave wins; 256-col octants
(top-8 of 256, 4 groups) give 1.6e-2 end-to-end rel err offline - under the
2e-2 gate but with too little margin vs the ~3e-3 baseline to ship; lc-pool
reformulations (leaky via 0.6u+0.4|u| with ACT Abs accumulators: 603us;
max-reduce from u16 in parallel with the leaky STT: 593us) both regress the
schedule despite lower DVE busy; per-tile incremental identity-mm accumulate
is neutral (-0.1us); gpsimd dma_gather (SBUF-source transpose mode) crashes
the runtime under TileContext - it needs manual .then_inc(dma_sem,16)
choreography the framework does not emit; the walrus GPSIMD engine
whitelist also rejects plain TensorTensor (even contiguous 2D max) and
free-axis tensor_reduce (bass allows only axis=C/XYZWC on gpsimd), so NO
generic vector op can be offloaded from DVE to Pool at all - DVE's ~425us
busy is the hard floor for this algorithm on this toolchain.)
"""
import sys

sys.path.insert(0, "/opt/trn_rl_repo")

import numpy as np

import concourse.bacc as bacc
import concourse.tile as tile
from concourse import mybir
from concourse.bass_utils import run_bass_kernel_spmd

F32 = mybir.dt.float32
F32R = mybir.dt.float32r
F16 = mybir.dt.float16
I32 = mybir.dt.int32
I16 = mybir.dt.int16

B, N, K = 16, 1024, 20
N_CORES = 8
ELS = B // N_CORES
CH_C = [3, 64, 64, 128]   # conv input channels (= prev layer out)
CH_O = [64, 64, 128, 256]
EMB = 1024
NT = N // 128
MMF = 512                 # psum-bank free-dim limit for one matmul
# quantization scales: |q| = |pd_ps|*S2 < 2^14 so (q<<10)|idx stays fp32-exact
# inside DVE Max8 (its int32 path runs through fp32). pd_ps absmax measured
# per layer: [23.8, 3.46, 0.745, 0.464], margin 1.25x.
S2_TAB = [2.0**14 / (1.25 * 23.8), 2.0**14 / (1.25 * 3.46),
          2.0**14 / (1.25 * 0.745), 2.0**14 / (1.25 * 0.464)]
# which layers pack ((q<<10)|idx) on gpsimd instead of DVE (DVE<->Pool balance)
POOL_PACK = [False, False, False, False]

AF = mybir.ActivationFunctionType
ALU = mybir.AluOpType
AX = mybir.AxisListType


def _mm(nc, out, lhsT, rhs, start, stop):
    fd = rhs.shape[-1]
    if fd <= MMF:
        nc.tensor.matmul(out=out, lhsT=lhsT, rhs=rhs, start=start, stop=stop)
        return
    for f0 in range(0, fd, MMF):
        f1 = min(f0 + MMF, fd)
        nc.tensor.matmul(out=out[:, f0:f1], lhsT=lhsT, rhs=rhs[:, f0:f1],
                         start=start, stop=stop)


def _r(ap):
    """fp32 passthrough (fp32r needs producer-side rounding; verifier rejects
    plain-fp32 producers feeding fp32r matmuls)."""
    return ap


def build_program():
    nc = bacc.Bacc("TRN2", target_bir_lowering=False, debug=False)

    x_in = nc.dram_tensor("x3", [ELS * 3, N], F32, kind="ExternalInput")
    wa_d, wc_d, wt_d = [], [], []
    for l in range(4):
        C, O = CH_C[l], CH_O[l]
        wdt = F32 if l == 0 else F16
        wa_d.append(nc.dram_tensor(f"wa{l}", [C, O], wdt, kind="ExternalInput"))
        wc_d.append(nc.dram_tensor(f"wc{l}", [C, O], wdt, kind="ExternalInput"))
        wt_d.append(nc.dram_tensor(f"wt{l}", [128, max(1, O // 128)],
                                   F32, kind="ExternalInput"))
    ident_d = nc.dram_tensor("ident16", [128, 128], F16, kind="ExternalInput")
    wlc_d = nc.dram_tensor("wlc16", [512, EMB], F16, kind="ExternalInput")
    lct_d = nc.dram_tensor("lct32", [128, 8], F32, kind="ExternalInput")
    wl0_d = nc.dram_tensor("wl0", [2049, 512], F32, kind="ExternalInput")
    wl1_d = nc.dram_tensor("wl1", [513, 256], F32, kind="ExternalInput")
    wow_d = nc.dram_tensor("wow", [257, 40], F32, kind="ExternalInput")
    out_d = nc.dram_tensor("out", [ELS, 40], F32, kind="ExternalOutput")

    with tile.TileContext(nc) as tc:
        with (
            tc.tile_pool(name="w", bufs=1) as wpool,
            tc.tile_pool(name="y", bufs=1) as ypool,
            tc.tile_pool(name="s1", bufs=1) as spool1,
            tc.tile_pool(name="s", bufs=2) as spool,
            tc.tile_pool(name="tk", bufs=3) as tkpool,
            tc.tile_pool(name="g", bufs=3) as gpool,
            tc.tile_pool(name="dr", bufs=2, space="DRAM") as dramp,
            tc.tile_pool(name="jit", bufs=4) as jitp,
        ):
            # ---------------- consts + resident weights ----------------
            ones_row = wpool.tile([1, N], F32, tag="ones_row")
            ones16 = wpool.tile([2, N], F16, tag="ones16")
            ones_col = wpool.tile([128, 1], F32, tag="ones_col")
            ones2 = wpool.tile([1, ELS], F32, tag="ones2")
            nc.vector.memset(ones_row[:], 1.0)
            nc.vector.memset(ones16[:], 1.0)
            nc.vector.memset(ones_col[:], 1.0)
            nc.vector.memset(ones2[:], 1.0)
            iota = wpool.tile([128, N], I32, tag="iota")
            nc.gpsimd.iota(iota[:], pattern=[[1, N]], base=0, channel_multiplier=0)
            s10 = wpool.tile([128, 1], I32, tag="s10")
            msk = wpool.tile([128, 1], I32, tag="msk")
            nc.vector.memset(s10[:], 10)
            nc.vector.memset(msk[:], 1023)
            ident = wpool.tile([128, 128], F16, tag="ident")
            nc.sync.dma_start(ident[:], ident_d.ap())

            x0_tiles = []
            for el in range(ELS):
                t = ypool.tile([3, N], F32, tag=f"x0_{el}", name=f"x0_{el}")
                nc.sync.dma_start(t[:], x_in.ap()[el * 3:(el + 1) * 3, :])
                x0_tiles.append(t)

            wa, wc, wt = [], [], []
            for l in range(4):
                C, O = CH_C[l], CH_O[l]
                wdt = F32 if l == 0 else F16
                ta = wpool.tile([C, O], wdt, tag=f"wa{l}")
                tcc = wpool.tile([C, O], wdt, tag=f"wc{l}")
                tt = wpool.tile([128, max(1, O // 128)], F32, tag=f"wt{l}")
                nc.sync.dma_start(ta[:], wa_d[l].ap())
                nc.sync.dma_start(tcc[:], wc_d[l].ap())
                nc.sync.dma_start(tt[:], wt_d[l].ap())
                wa.append(ta); wc.append(tcc); wt.append(tt)

            wlc = []
            lc_rows = [(0, 64), (64, 128), (128, 256), (256, 384), (384, 512)]
            for kc, (r0, r1) in enumerate(lc_rows):
                twl = wpool.tile([r1 - r0, EMB], F16, tag=f"wlc{kc}")
                nc.sync.dma_start(twl[:], wlc_d.ap()[r0:r1, :])
                wlc.append(twl)
            lct = wpool.tile([128, 8], F32, tag="lct")
            nc.sync.dma_start(lct[:], lct_d.ap())

            # fp16 feature tiles per el (each base-partition 0: PE operands)
            h0 = [wpool.tile([64, N], F16, tag=f"h0_{el}", name=f"h0_{el}")
                  for el in range(ELS)]
            h1 = [wpool.tile([64, N], F16, tag=f"h1_{el}", name=f"h1_{el}")
                  for el in range(ELS)]
            h2 = [wpool.tile([128, N], F16, tag=f"h2_{el}", name=f"h2_{el}")
                  for el in range(ELS)]
            h3a = [wpool.tile([128, N], F16, tag=f"h3a{el}", name=f"h3a{el}")
                   for el in range(ELS)]
            h3b = [wpool.tile([128, N], F16, tag=f"h3b{el}", name=f"h3b{el}")
                   for el in range(ELS)]

            maxes = ypool.tile([128, NT, ELS], F32, tag="maxes")
            sums = ypool.tile([128, NT, ELS], F32, tag="sums")

            with (
                tc.tile_pool(name="pspd", bufs=2, space="PSUM") as pspd,
                tc.tile_pool(name="psmm", bufs=2, space="PSUM") as psmm,
            ):
                for l in range(4):
                    C, O = CH_C[l], CH_O[l]
                    packed_lay = (O == 64)
                    nch = 1 if packed_lay else O // 128
                    S2 = S2_TAB[l]

                    # per-el knn feature source (fp16 except l0)
                    if l == 0:
                        feats = [x0_tiles[el][:] for el in range(ELS)]
                    elif l == 1:
                        feats = [h0[el][:] for el in range(ELS)]
                    elif l == 2:
                        feats = [h1[el][:] for el in range(ELS)]
                    else:
                        feats = [h2[el][:] for el in range(ELS)]

                    # wrapped+replicated idx tiles
                    if packed_lay:
                        iwt = [spool1.tile([128, NT * 160], I16, tag="iw0",
                                           name=f"iw_l{l}")]
                    else:
                        iwt = [spool1.tile([128, NT * 160], I16, tag=f"iw{el}",
                                           name=f"iw_l{l}_{el}")
                               for el in range(ELS)]

                    # per-el prep: xx = sum_c x_c^2 ; nxxh rows
                    nxx_rows = []
                    for el in range(ELS):
                        xf = feats[el]
                        xsq = spool1.tile([C, N], F32, tag=f"xsq{el}",
                                          name=f"xsq{el}")
                        nc.scalar.activation(out=xsq[:], in_=xf, func=AF.Square)
                        xx_ps = psmm.tile([1, N], F32, tag="mm")
                        _mm(nc, xx_ps[:], _r(ones_col[0:C, :]), _r(xsq[:]),
                            True, True)
                        if l == 0:
                            nxxh = spool1.tile([1, N], F32, tag=f"nxxh{el}",
                                               name=f"nxxh{el}")
                            nc.scalar.activation(out=nxxh[:], in_=xx_ps[:],
                                                 func=AF.Copy, scale=-0.5)
                            nxx_rows.append(nxxh)
                        else:
                            nxxh32 = spool1.tile([1, N], F32, tag=f"nxh32_{el}",
                                                 name=f"nxh32_{el}")
                            nc.scalar.activation(out=nxxh32[:], in_=xx_ps[:],
                                                 func=AF.Copy, scale=-0.5)
                            nxx2 = spool1.tile([2, N], F16, tag=f"nxx2_{el}",
                                               name=f"nxx2_{el}")
                            nc.scalar.activation(out=nxx2[0:1, :], in_=xx_ps[:],
                                                 func=AF.Copy, scale=-0.5)
                            resid = spool1.tile([1, N], F16, tag="resid")
                            nc.vector.tensor_tensor(out=resid[:],
                                                    in0=nxxh32[:],
                                                    in1=nxx2[0:1, :],
                                                    op=ALU.subtract)
                            nc.sync.dma_start(nxx2[1:2, :], resid[:])
                            nxx_rows.append(nxx2)

                    flats = [dramp.tile([NT * 128, K], I16, tag=f"idxflat{el}",
                                        name=f"fl{l}_{el}")
                             for el in range(ELS)]
                    for t in range(NT):
                        for el in range(ELS):
                            xf = feats[el]
                            pd_ps = pspd.tile([128, N], F32, tag="pd")
                            if l == 0:
                                _mm(nc, pd_ps[:],
                                    _r(xf[:, t * 128:(t + 1) * 128]), _r(xf),
                                    True, False)
                                _mm(nc, pd_ps[:],
                                    _r(ones_row[:, t * 128:(t + 1) * 128]),
                                    _r(nxx_rows[el][:]), False, True)
                            else:
                                _mm(nc, pd_ps[:], xf[:, t * 128:(t + 1) * 128],
                                    xf, True, False)
                                _mm(nc, pd_ps[:],
                                    ones16[:, t * 128:(t + 1) * 128],
                                    nxx_rows[el][:], False, True)
                            q = tkpool.tile([128, N], I32, tag="q")
                            nc.scalar.activation(out=q[:], in_=pd_ps[:],
                                                 func=AF.Copy, scale=S2)
                            pk = tkpool.tile([128, N], I32, tag="pk")
                            pack_eng = (nc.gpsimd if POOL_PACK[l] else
                                        nc.vector)
                            pack_eng.scalar_tensor_tensor(
                                out=pk[:], in0=q[:], scalar=s10[:], in1=iota[:],
                                op0=ALU.arith_shift_left, op1=ALU.bitwise_or)
                            cand = tkpool.tile([128, 64], I32, tag="cand")
                            for s in range(8):
                                nc.vector.max(out=cand[:, 8 * s:8 * (s + 1)],
                                              in_=pk[:, 128 * s:128 * (s + 1)])
                            top = tkpool.tile([128, 24], I32, tag="top")
                            nc.vector.max(out=top[:, 0:8], in_=cand[:])
                            nc.vector.match_replace(
                                out=cand[:], in_to_replace=top[:, 0:8],
                                in_values=cand[:], imm_value=-2.0e9)
                            nc.vector.max(out=top[:, 8:16], in_=cand[:])
                            nc.vector.match_replace(
                                out=cand[:], in_to_replace=top[:, 8:16],
                                in_values=cand[:], imm_value=-2.0e9)
                            nc.vector.max(out=top[:, 16:24], in_=cand[:])
                            ixt = tkpool.tile([128, 24], I32, tag="ix")
                            nc.vector.tensor_scalar(
                                out=ixt[:], in0=top[:], scalar1=msk[:],
                                scalar2=None, op0=ALU.bitwise_and)
                            ix16 = tkpool.tile([128, 24], I16, tag="ix16")
                            nc.vector.tensor_copy(ix16[:], ixt[:])
                            # e-order dump -> wrapped strided read -> log2 repl
                            iw = iwt[0] if packed_lay else iwt[el]
                            p_base = 64 * el if packed_lay else 0
                            nrep = 4 if packed_lay else 8
                            flat = flats[el]
                            c0, c1 = t * 160, (t + 1) * 160
                            nc.sync.dma_start(flat[t * 128:(t + 1) * 128, :],
                                              ix16[:, 0:K])
                            wsrc = (flat[t * 128:(t + 1) * 128, :]
                                    .rearrange("p r -> (p r)")
                                    .rearrange("(s w) -> w s", w=16))
                            nc.sync.dma_start(iw[p_base:p_base + 16, c0:c1], wsrc)
                            blk = 16
                            while blk < 16 * nrep:
                                nc.sync.dma_start(
                                    iw[p_base + blk:p_base + 2 * blk, c0:c1],
                                    iw[p_base:p_base + blk, c0:c1])
                                blk *= 2

                    # ---- convs + gather + m-accum + bias + leaky ----
                    def conv_chunk(a_sb, c_ps, iw, wt_bias, lrelus):
                        """gather a_sb (128ch fp32), grouped max -> m16,
                        accumulate m into c_ps on PE, ACT bias -> u16 fp16,
                        gpsimd leaky -> h dsts. lrelus: (hdst, p0, p1)."""
                        m16 = spool.tile([128, N], F16, tag="m16")
                        for t in range(NT):
                            g = gpool.tile([128, 2560], F32, tag="gath")
                            nc.gpsimd.ap_gather(
                                out_ap=g[:], in_ap=a_sb[:],
                                idxs_ap=iw[:, t * 160:(t + 1) * 160],
                                channels=128, num_elems=N, d=1, num_idxs=2560)
                            nc.vector.tensor_reduce(
                                out=m16[:, t * 128:(t + 1) * 128],
                                in_=g[:].rearrange("p (i r) -> p i r", r=K),
                                axis=AX.X, op=ALU.max)
                        _mm(nc, c_ps[:], ident[:], m16[:], False, True)
                        u16 = spool.tile([128, N], F16, tag="u16")
                        nc.scalar.activation(out=u16[:], in_=c_ps[:],
                                             func=AF.Identity,
                                             bias=wt_bias)
                        for hdst, p0, p1 in lrelus:
                            nc.vector.scalar_tensor_tensor(
                                out=hdst, in0=u16[p0:p1, :], scalar=0.2,
                                in1=u16[p0:p1, :], op0=ALU.mult, op1=ALU.max)

                    if packed_lay:
                        a_sb = spool.tile([128, N], F32, tag="asb")
                        c_ps = psmm.tile([128, N], F32, tag="mm")
                        for el in range(ELS):
                            a_ps = pspd.tile([128, N], F32, tag="pd")
                            if l == 0:
                                _mm(nc, a_ps[0:64, :],
                                    _r(wa[l][:, 0:O]), _r(feats[el]), True, True)
                                _mm(nc, c_ps[64 * el:64 * (el + 1), :],
                                    _r(wc[l][:, 0:O]), _r(feats[el]), True, False)
                            else:
                                _mm(nc, a_ps[0:64, :],
                                    wa[l][:, 0:O], feats[el], True, True)
                                _mm(nc, c_ps[64 * el:64 * (el + 1), :],
                                    wc[l][:, 0:O], feats[el], True, False)
                            nc.scalar.activation(
                                out=a_sb[64 * el:64 * (el + 1), :],
                                in_=a_ps[0:64, :], func=AF.Copy)
                        ydst = h0 if l == 0 else h1
                        conv_chunk(a_sb, c_ps, iwt[0], wt[l][:, 0:1],
                                   [(ydst[el][:], 64 * el, 64 * (el + 1))
                                    for el in range(ELS)])
                    else:
                        for el in range(ELS):
                            for ch in range(nch):
                                o0, o1 = ch * 128, (ch + 1) * 128
                                a_sb = spool.tile([128, N], F32, tag="asb")
                                a_ps = pspd.tile([128, N], F32, tag="pd")
                                _mm(nc, a_ps[:], wa[l][:, o0:o1], feats[el],
                                    True, True)
                                nc.scalar.activation(out=a_sb[:], in_=a_ps[:],
                                                     func=AF.Copy)
                                c_ps = psmm.tile([128, N], F32, tag="mm")
                                _mm(nc, c_ps[:], wc[l][:, o0:o1], feats[el],
                                    True, False)
                                dst = (h2[el][:] if l == 2 else
                                       (h3a[el][:] if ch == 0 else h3b[el][:]))
                                conv_chunk(a_sb, c_ps, iwt[el],
                                           wt[l][:, ch:ch + 1],
                                           [(dst, 0, 128)])

                # ================= lc conv + pooling =================
                for el in range(ELS):
                    rhs_chunks = [h0[el][:], h1[el][:], h2[el][:],
                                  h3a[el][:], h3b[el][:]]
                    for mt in range(8):
                        u_ps = pspd.tile([128, N], F32, tag="pd")
                        for kc in range(5):
                            _mm(nc, u_ps[:], wlc[kc][:, mt * 128:(mt + 1) * 128],
                                rhs_chunks[kc], kc == 0, kc == 4)
                        u16 = spool.tile([128, N], F16, tag="u16")
                        nc.scalar.activation(out=u16[:], in_=u_ps[:],
                                             func=AF.Identity,
                                             bias=lct[:, mt:mt + 1])
                        y5 = spool.tile([128, N], F16, tag="y5")
                        nc.vector.scalar_tensor_tensor(
                            out=y5[:], in0=u16[:], scalar=0.2, in1=u16[:],
                            op0=ALU.mult, op1=ALU.max,
                            accum_out=sums[:, mt:mt + 1, el:el + 1]
                            .rearrange("p a b -> p (a b)"))
                        nc.vector.tensor_reduce(out=maxes[:, mt:mt + 1, el:el + 1],
                                                in_=y5[:], axis=AX.X, op=ALU.max)

            # ================= FC head (els together as F=ELS) =================
            with tc.tile_pool(name="psfc", bufs=1, space="PSUM") as psfc:
                l0ps = [psfc.tile([128, ELS], F32, tag=f"fc{mt}", name=f"fc{mt}")
                        for mt in range(4)]
                for kc in range(17):
                    r0, r1 = (kc * 128, (kc + 1) * 128) if kc < 16 else (2048, 2049)
                    wj = jitp.tile([r1 - r0, 512], F32, tag="wj0")
                    nc.sync.dma_start(wj[:], wl0_d.ap()[r0:r1, :])
                    if kc < 8:
                        rhs = maxes[:, kc:kc + 1, :].rearrange("p a b -> p (a b)")
                    elif kc < 16:
                        rhs = sums[:, kc - 8:kc - 7, :].rearrange("p a b -> p (a b)")
                    else:
                        rhs = ones2[:]
                    for mt in range(4):
                        nc.tensor.matmul(out=l0ps[mt][:],
                                         lhsT=wj[:, mt * 128:(mt + 1) * 128],
                                         rhs=rhs, start=kc == 0, stop=kc == 16)
                y6 = ypool.tile([128, 4 * ELS], F32, tag="y6")
                y6v = y6[:].rearrange("p (a b) -> p a b", a=4)
                for mt in range(4):
                    u = spool.tile([128, ELS], F32, tag="fcu")
                    zs = spool.tile([128, ELS], F32, tag="fczs")
                    nc.vector.tensor_scalar_mul(zs[:], l0ps[mt][:], 0.2)
                    nc.vector.tensor_tensor(out=u[:], in0=l0ps[mt][:], in1=zs[:],
                                            op=ALU.max)
                    nc.vector.tensor_copy(y6v[:, mt:mt + 1, :],
                                          u[:].rearrange("p (a b) -> p a b", a=1))
                l1ps = [psfc.tile([128, ELS], F32, tag=f"fd{mt}", name=f"fd{mt}")
                        for mt in range(2)]
                for kc in range(5):
                    r0, r1 = (kc * 128, (kc + 1) * 128) if kc < 4 else (512, 513)
                    wj = jitp.tile([r1 - r0, 256], F32, tag="wj1")
                    nc.sync.dma_start(wj[:], wl1_d.ap()[r0:r1, :])
                    rhs = (y6v[:, kc:kc + 1, :].rearrange("p a b -> p (a b)")
                           if kc < 4 else ones2[:])
                    for mt in range(2):
                        nc.tensor.matmul(out=l1ps[mt][:],
                                         lhsT=wj[:, mt * 128:(mt + 1) * 128],
                                         rhs=rhs, start=kc == 0, stop=kc == 4)
                y7 = ypool.tile([128, 2 * ELS], F32, tag="y7")
                y7v = y7[:].rearrange("p (a b) -> p a b", a=2)
                for mt in range(2):
                    u = spool.tile([128, ELS], F32, tag="fcu")
                    zs = spool.tile([128, ELS], F32, tag="fczs")
                    nc.vector.tensor_scalar_mul(zs[:], l1ps[mt][:], 0.2)
                    nc.vector.tensor_tensor(out=u[:], in0=l1ps[mt][:], in1=zs[:],
                                            op=ALU.max)
                    nc.vector.tensor_copy(y7v[:, mt:mt + 1, :],
                                          u[:].rearrange("p (a b) -> p a b", a=1))
                ops_ = psfc.tile([ELS, 40], F32, tag="fcout")
                for kc in range(3):
                    if kc < 2:
                        lhsT = y7v[:, kc:kc + 1, :].rearrange("p a b -> p (a b)")
                        wj = jitp.tile([128, 40], F32, tag="wjo")
                        nc.sync.dma_start(wj[:], wow_d.ap()[kc * 128:(kc + 1) * 128, :])
                    else:
                        lhsT = ones2[:]
                        wj = jitp.tile([1, 40], F32, tag="wjob")
                        nc.sync.dma_start(wj[:], wow_d.ap()[256:257, :])
                    nc.tensor.matmul(out=ops_[:], lhsT=lhsT, rhs=wj[:],
                                     start=kc == 0, stop=kc == 2)
                osb = spool.tile([ELS, 40], F32, tag="osb")
                nc.scalar.activation(out=osb[:], in_=ops_[:], func=AF.Copy)
                nc.sync.dma_start(out_d.ap(), osb[:])

    nc.compile()
    return nc


def _fold_weights(i):
    out = {}
    for l in range(4):
        C = CH_C[l]
        w = np.asarray(i[f"c{l}_w"], np.float64)
        b = np.asarray(i[f"c{l}_b"], np.float64)
        g = np.asarray(i[f"c{l}_g"], np.float64)
        be = np.asarray(i[f"c{l}_be"], np.float64)
        m = np.asarray(i[f"c{l}_m"], np.float64)
        v = np.asarray(i[f"c{l}_v"], np.float64)
        s = g / np.sqrt(v + 1e-5)
        w1, w2 = w[:, :C], w[:, C:]
        dt = np.float32 if l == 0 else np.float16
        out[f"wa{l}"] = np.ascontiguousarray((s[:, None] * w1).T).astype(dt)
        out[f"wc{l}"] = np.ascontiguousarray((s[:, None] * (w2 - w1)).T).astype(dt)
        tvec = (s * b + be - s * m).astype(np.float32)
        ow = tvec.shape[0]
        if ow < 128:
            tvec = np.tile(tvec, 128 // ow)  # packed layers: bias for both els
        out[f"wt{l}"] = np.ascontiguousarray(
            tvec.reshape(max(1, ow // 128), 128).T)
    out["ident16"] = np.eye(128, dtype=np.float16)
    s = np.asarray(i["lc_g"], np.float64) / np.sqrt(np.asarray(i["lc_v"], np.float64) + 1e-5)
    t = s * np.asarray(i["lc_b"], np.float64) + np.asarray(i["lc_be"], np.float64) \
        - s * np.asarray(i["lc_m"], np.float64)
    out["wlc16"] = np.ascontiguousarray(
        (s[:, None] * np.asarray(i["lc_w"], np.float64)).T).astype(np.float16)
    out["lct32"] = np.ascontiguousarray(
        t.astype(np.float32).reshape(8, 128).T)
    s = np.asarray(i["l0_g"], np.float64) / np.sqrt(np.asarray(i["l0_v"], np.float64) + 1e-5)
    t = np.asarray(i["l0_be"], np.float64) - s * np.asarray(i["l0_m"], np.float64)
    w = s[:, None] * np.asarray(i["l0_w"], np.float64)
    w[:, 1024:] /= 1024.0
    out["wl0"] = np.ascontiguousarray(np.concatenate([w.T, t[None, :]], 0)).astype(np.float32)
    s = np.asarray(i["l1_g"], np.float64) / np.sqrt(np.asarray(i["l1_v"], np.float64) + 1e-5)
    t = s * np.asarray(i["l1_b"], np.float64) + np.asarray(i["l1_be"], np.float64) \
        - s * np.asarray(i["l1_m"], np.float64)
    out["wl1"] = np.ascontiguousarray(
        np.concatenate([(s[:, None] * np.asarray(i["l1_w"], np.float64)).T,
                        t[None, :]], 0)).astype(np.float32)
    out["wow"] = np.ascontiguousarray(
        np.concatenate([np.asarray(i["ow"], np.float32).T,
                        np.asarray(i["ob"], np.float32)[None, :]], 0))
    return out


_NC_CACHE = {}


def get_program(debug=False):
    if debug not in _NC_CACHE:
        _NC_CACHE[debug] = build_program()
    return _NC_CACHE[debug]


def make_in_maps(inputs):
    folded = _fold_weights(inputs)
    x = np.asarray(inputs["x"], np.float32)
    in_maps = []
    for c in range(N_CORES):
        m = dict(folded)
        xs = x[c * ELS:(c + 1) * ELS]                       # (ELS, 1024, 3)
        m["x3"] = np.ascontiguousarray(
            xs.transpose(0, 2, 1).reshape(ELS * 3, N))
        in_maps.append(m)
    return in_maps


def kernel(**inputs) -> np.ndarray:
    nc = get_program(False)
    in_maps = make_in_maps(inputs)
    res = run_bass_kernel_spmd(nc, in_maps, list(range(N_CORES)))
    outs = [res.results[c]["out"] for c in range(N_CORES)]
    return np.concatenate(outs, 0).astype(np.float32)
